# revision 1
# baseline (speedup 1.0000x reference)
"""Trainium2 Bass kernel for nn_DistanceLoss (retrieval_knn, 5-way 5-shot).

Computation (per reference):
    q  = relu(queries.flat @ W.T + b)          [5600, 1024]
    se = relu(support.flat @ W.T + b)          [1400, 1024]
    d2 = q_sq + s_sq - 2 q @ se.T              [5600, 1400]
    out[q, c] = -mean_t min_{j in class c} sqrt(relu(d2))

Sharding (8 cores):
  - data-parallel over queries: 13 queries (728 rows) per core (padded 100->104)
  - support projection sharded by support rows (175 rows/core), AllGathered
  - min over class = contiguous 280-col chunk (support rows class-sorted on host)

Layout: "transposed activations" — all matmul operands keep the contraction
dim on partitions. Host pre-transposes Q/W/S and supplies them in fp16
(accumulation is fp32 in PSUM; final rel err ~5e-5). Bias is folded in as an
extra contraction row. q_sq/s_sq are folded into the distance matmul as
fp16 hi+lo split pairs (4 rank-1 terms), so PSUM holds d2 directly at fp32
precision. min(sqrt(relu(x))) == sqrt(relu(min(x))), so the min reduction
runs on raw d2 and sqrt touches only [rows, 5].

Schedule: one fused k-sweep computes the support projection and most of the
query projection sharing W tiles; the AllGather of se.T fires right after,
and the remaining query m-tiles run as a tail sweep that keeps the PE busy
(and HAM-warm) while the collective is in flight.
"""

import os
import sys

if "/opt/trn_rl_repo" not in sys.path:
    sys.path.insert(0, "/opt/trn_rl_repo")

import numpy as np

import concourse.bacc as bacc
import concourse.mybir as mybir
import concourse.tile as tile
from concourse.bass_utils import run_bass_kernel_spmd

WAY, SHOT, T = 5, 5, 56
D_IN, D_OUT = 6144, 1024
N_Q, N_S = 100, 25
N_CORES = 8
QPC = 13                 # queries per core (104 padded)
RPC = QPC * T            # 728 query rows per core
NQR = N_CORES * RPC      # 5824 padded query rows
NSR = N_S * T            # 1400 support rows
SPC = NSR // N_CORES     # 175 support rows per core
KT = D_IN // 128         # 48 full k-tiles
GK = 8                   # max k-tiles per group
GROUPS = [1, 1, 2, 4, 8, 8, 12, 12]  # k-tiles per group (sum 48): small
    # first groups so PE starts early; big last group so the last group's
    # query matmuls (post-collective-trigger) cover the AllGather latency
QM_MAIN = 8              # all query m-tiles run in the shared sweep
NCH = RPC // 2           # 364: query-row matmul chunk
CLS = NSR // WAY         # 280 columns per class
MT = (RPC + 127) // 128  # 6 row tiles (5x128 + 88)
SMW = (128, SPC - 128)   # support row-tile widths (128, 47)
SPCP = 176               # SPC padded: allgather payload [1026,176] fp16 is
                         # a 64B multiple (the collective hangs otherwise)

f32 = mybir.dt.float32
f16 = mybir.dt.float16
AF = mybir.ActivationFunctionType
ALU = mybir.AluOpType
AX = mybir.AxisListType

_MODE = os.environ.get("KERNEL_MODE", "full")


def _build_nc():
    nc = bacc.Bacc("TRN2", target_bir_lowering=False, debug=False,
                   num_devices=N_CORES)
    qT = nc.dram_tensor("qT", [D_IN + 1, RPC], f16, kind="ExternalInput")
    wT = nc.dram_tensor("wT", [D_IN + 1, D_OUT], f16, kind="ExternalInput")
    sT = nc.dram_tensor("sT", [D_IN + 1, SPC], f16, kind="ExternalInput")
    mmask = nc.dram_tensor("mmask", [MT * 128, QPC], f32, kind="ExternalInput")
    ident = nc.dram_tensor("ident", [128, 128], f32, kind="ExternalInput")
    onesd = nc.dram_tensor("onesd", [128, NSR], f16, kind="ExternalInput")
    out = nc.dram_tensor("out", [QPC, WAY], f32, kind="ExternalOutput")

    with tile.TileContext(nc) as tc:
        _body(tc, nc, qT, wT, sT, mmask, ident, onesd, out)
    nc.finalize()
    return nc


def _body(tc, nc, qT, wT, sT, mmask, ident, onesd, out):
    persist_ctx = tc.tile_pool(name="persist", bufs=1)
    persist = persist_ctx.__enter__()

    def ptile(shape, name, dtype=f32):
        return persist.tile(shape, dtype, tag=name, name=name)

    # ---- persistent tiles (live across phases) ----
    # fp32 k-group accumulators
    qacc = [ptile([128, RPC], f"qacc{m}") for m in range(8)]
    sacc = [ptile([SMW[sm], D_OUT], f"sacc{sm}") for sm in range(2)]
    # fp16 operands for the distance matmul
    qacc16 = [ptile([128, RPC], f"qacc16_{m}", f16) for m in range(8)]
    ssq_cols = (ptile([128, 1], "ssq0"), ptile([SMW[1], 1], "ssq1"))

    ident_t = ptile([128, 128], "ident_t")
    ones_col = ptile([128, 1], "ones_col", f16)
    qsq_cols = [ptile([128, 1], f"qsqc{mt}") for mt in range(MT)]
    ssq2 = ptile([2, NSR], "ssq2", f16)
    ones2 = ptile([2, NSR], "ones2", f16)
    ssq_hi_row = ptile([1, SPC], "ssq_hi_row", f16)
    ssq_lo_row = ptile([1, SPC], "ssq_lo_row", f16)
    mins = [ptile([128, WAY], f"mins{mt}") for mt in range(MT)]

    # ragged contraction row (bias / ones)
    wr = ptile([1, D_OUT], "wr", f16)
    qr = ptile([1, RPC], "qr", f16)
    sr = ptile([1, SPC], "sr", f16)

    def emit_preamble():
        # constants not needed before ~85us; emitted mid-sweep-A so their
        # small/strided DMAs don't delay the first group loads
        nc.sync.dma_start(out=ident_t[:], in_=ident[:])
        nc.sync.dma_start(out=ones_col[:], in_=onesd[:, 0:1])
        nc.sync.dma_start(out=ones2[:], in_=onesd[0:2, :])
        nc.sync.dma_start(out=wr[:], in_=wT[D_IN:D_IN + 1, :])
        nc.sync.dma_start(out=qr[:], in_=qT[D_IN:D_IN + 1, :])
        nc.sync.dma_start(out=sr[:], in_=sT[D_IN:D_IN + 1, :])

    # ---- q_sq infrastructure: squares persist until the q_sq-column
    # matmuls after the tail sweep ----
    sqt = [ptile([128, RPC], f"sq{m}", f16) for m in range(8)]

    def emit_qsq(m):
        # relu + fp16 cast, then square (summed later per row-tile)
        nc.vector.tensor_scalar_max(qacc16[m][:], qacc[m][:], 0.0)
        nc.scalar.activation(sqt[m][:], qacc16[m][:], AF.Square)

    # ---- sweep-B first-group preload (loaded during sweep A so the PE
    # transitions between sweeps without a DMA wait) ----
    pre_ctx = tc.tile_pool(name="preload", bufs=1)
    prepool = pre_ctx.__enter__()
    wpre = prepool.tile([128, 2, D_OUT], f16, tag="wpre", name="wpre")
    qpre = prepool.tile([128, 2, RPC], f16, tag="qpre", name="qpre")

    def emit_preload():
        nc.sync.dma_start(
            out=wpre[:],
            in_=wT[0:256, :].rearrange("(g p) d -> p g d", p=128))
        nc.sync.dma_start(
            out=qpre[:],
            in_=qT[0:256, :].rearrange("(g p) d -> p g d", p=128))

    # ---- allgather buffers (the collective fires inside the last k-group,
    # before that group's query matmuls, to start it as early as possible) ----
    dram_ctx = tc.tile_pool(name="dram", bufs=1, space="DRAM")
    dram = dram_ctx.__enter__()
    ag_in = dram.tile([D_OUT + 2, SPCP], f16, tag="ag_in", name="ag_in")
    ag_out = dram.tile([N_CORES, D_OUT + 2, SPCP], f16, tag="ag_out",
                       name="ag_out",
                       addr_space="Local" if _MODE == "nocc" else "Shared")

    # ---- sweep A: support projection only (k-grouped, W+S streamed).
    # Finishing support early lets the AllGather fire at ~55us and hide
    # completely under the query sweep, robust to collective-time variance.
    with (
        tc.tile_pool(name="wspool", bufs=3) as wspool,
        tc.tile_pool(name="sspool", bufs=3) as sspool,
        tc.tile_pool(name="ps", bufs=4, space="PSUM") as pspool,
        tc.tile_pool(name="ssq_scratch", bufs=2) as scratch_pool,
        tc.tile_pool(name="setl", bufs=1) as setl_pool,
        tc.tile_pool(name="ptr", bufs=4, space="PSUM") as ptr_pool,
    ):
        def emit_support_gather():
            # transpose the scaled local se into se.T columns, ship to DRAM,
            # and fire the AllGather
            for j in range(8):
                setl = setl_pool.tile([128, SPC], f16, tag=f"setl{j}",
                                      name=f"setl{j}")
                for sm in range(2):
                    mw = SMW[sm]
                    ptr = ptr_pool.tile([128, 128], f32, tag="ptr",
                                        name="ptr")
                    nc.tensor.transpose(
                        ptr[:, :mw],
                        sacc[sm][:, j * 128:(j + 1) * 128],
                        ident_t[:mw, :mw],
                    )
                    nc.vector.tensor_copy(setl[:, sm * 128:sm * 128 + mw],
                                          ptr[:, :mw])
                nc.sync.dma_start(out=ag_in[j * 128:(j + 1) * 128, 0:SPC],
                                  in_=setl[:])
            for sm in range(2):
                mw = SMW[sm]
                ptr = ptr_pool.tile([128, 128], f32, tag="ptr", name="ptr")
                nc.tensor.transpose(ptr[:1, :mw], ssq_cols[sm][:mw, :],
                                    ident_t[:mw, :mw])
                osl = slice(sm * 128, sm * 128 + mw)
                nc.vector.tensor_copy(ssq_hi_row[:, osl], ptr[:1, :mw])
                nc.vector.tensor_sub(ssq_lo_row[:, osl], ptr[:1, :mw],
                                     ssq_hi_row[:, osl])
            nc.sync.dma_start(out=ag_in[D_OUT:D_OUT + 1, 0:SPC],
                              in_=ssq_hi_row[:])
            nc.sync.dma_start(out=ag_in[D_OUT + 1:D_OUT + 2, 0:SPC],
                              in_=ssq_lo_row[:])
            if _MODE == "nocc":
                for c in range(N_CORES):
                    nc.sync.dma_start(out=ag_out[c], in_=ag_in[:])
            else:
                nc.gpsimd.collective_compute(
                    "AllGather",
                    ALU.bypass,
                    replica_groups=[list(range(N_CORES))],
                    ins=[ag_in[:]],
                    outs=[ag_out[:]],
                )

        SGROUPS = [1, 1, 2, 2, 4, 6, 8, 8, 8, 8]
        kstart = 0
        for g in range(len(SGROUPS)):
            kts = list(range(kstart, kstart + SGROUPS[g]))
            kstart += SGROUPS[g]
            last = g == len(SGROUPS) - 1
            nk = len(kts)
            k0 = kts[0]
            if g == 2:
                emit_preamble()
                emit_preload()
            wg = wspool.tile([128, nk, D_OUT], f16, tag="ws", name=f"ws{g}")
            nc.sync.dma_start(
                out=wg[:],
                in_=wT[k0 * 128:k0 * 128 + nk * 128, :]
                .rearrange("(g p) d -> p g d", p=128))
            sg = sspool.tile([128, nk, SPC], f16, tag="ss", name=f"ss{g}")
            nc.sync.dma_start(
                out=sg[:],
                in_=sT[k0 * 128:k0 * 128 + nk * 128, :]
                .rearrange("(g p) d -> p g d", p=128))

            for sm in range(2):
                mw = SMW[sm]
                msl = slice(sm * 128, sm * 128 + mw)
                for n in range(2):
                    nsl = slice(n * 512, (n + 1) * 512)
                    pst = pspool.tile([128, 512], f32, tag="ps", name="pst")
                    for i, kt in enumerate(kts):
                        nc.tensor.matmul(
                            pst[:mw, :],
                            sg[:, i, msl],
                            wg[:, i, nsl],
                            start=(i == 0),
                            stop=(i == nk - 1 and not last),
                        )
                    if last:
                        nc.tensor.matmul(
                            pst[:mw, :],
                            sr[:, msl],
                            wr[:, nsl],
                            start=False, stop=True,
                        )
                    if g == 0:
                        nc.vector.tensor_copy(sacc[sm][:, nsl], pst[:mw, :])
                    else:
                        nc.vector.tensor_add(sacc[sm][:, nsl],
                                             sacc[sm][:, nsl], pst[:mw, :])

            if last:
                # support epilogue: sacc = -2*relu(raw) = min(-2*raw, 0);
                # s_sq = sum(relu(raw)^2) = sum((0.5*sacc)^2) via ACT accum
                for sm in range(2):
                    mw = SMW[sm]
                    nc.vector.tensor_scalar(sacc[sm][:], sacc[sm][:],
                                            -2.0, 0.0, ALU.mult, ALU.min)
                    sc = scratch_pool.tile([128, D_OUT], f32, tag="ssq_sc",
                                           name="ssq_sc")
                    nc.scalar.activation(sc[:mw, :], sacc[sm][:], AF.Square,
                                         scale=0.5,
                                         accum_out=ssq_cols[sm][:mw, :])
                emit_support_gather()

    # ---- sweep B: query projection (all m-tiles), overlaps the AllGather
    with (
        tc.tile_pool(name="wpool", bufs=2) as wpool,
        tc.tile_pool(name="qpool", bufs=2) as qpool,
        tc.tile_pool(name="pq", bufs=4, space="PSUM") as pqpool,
    ):
        QGROUPS = [2, 4, 6, 8, 8, 8, 8, 4]
        kstart = 0
        for g in range(len(QGROUPS)):
            kts = list(range(kstart, kstart + QGROUPS[g]))
            kstart += QGROUPS[g]
            last = g == len(QGROUPS) - 1
            nk = len(kts)
            k0 = kts[0]
            if g == 0:
                wg, qg = wpre, qpre
            else:
                wg = wpool.tile([128, nk, D_OUT], f16, tag="w", name=f"w{g}")
                nc.sync.dma_start(
                    out=wg[:],
                    in_=wT[k0 * 128:k0 * 128 + nk * 128, :]
                    .rearrange("(g p) d -> p g d", p=128))
                qg = qpool.tile([128, nk, RPC], f16, tag="q", name=f"q{g}")
                nc.sync.dma_start(
                    out=qg[:],
                    in_=qT[k0 * 128:k0 * 128 + nk * 128, :]
                    .rearrange("(g p) d -> p g d", p=128))

            for m in range(8):
                msl = slice(m * 128, (m + 1) * 128)
                for n in range(2):
                    nsl = slice(n * NCH, (n + 1) * NCH)
                    pqt = pqpool.tile([128, NCH], f32, tag="pq", name="pqt")
                    for i, kt in enumerate(kts):
                        nc.tensor.matmul(
                            pqt[:],
                            wg[:, i, msl],
                            qg[:, i, nsl],
                            start=(i == 0),
                            stop=(i == nk - 1 and not last),
                        )
                    if last:
                        nc.tensor.matmul(
                            pqt[:],
                            wr[:, msl],
                            qr[:, nsl],
                            start=False, stop=True,
                        )
                    if g == 0:
                        nc.vector.tensor_copy(qacc[m][:, nsl], pqt[:])
                    else:
                        nc.vector.tensor_add(qacc[m][:, nsl],
                                             qacc[m][:, nsl], pqt[:])
            if last:
                for m in range(8):
                    emit_qsq(m)

    # ---- q_sq columns: qsq_col[mt][r] = sum_dout q^2, via sq.T @ ones ----
    with tc.tile_pool(name="pqsqc", bufs=2, space="PSUM") as pqsqc:
        for mt in range(MT):
            mw = min(128, RPC - mt * 128)
            msl = slice(mt * 128, mt * 128 + mw)
            pq1 = pqsqc.tile([128, 1], f32, tag="pqsqc", name="pqsqc")
            for j in range(8):
                nc.tensor.matmul(pq1[:mw, :], sqt[j][:, msl], ones_col[:],
                                 start=(j == 0), stop=(j == 7))
            nc.vector.tensor_copy(qsq_cols[mt][:mw, :], pq1[:mw, :])

    # ---- phase 2: distance + per-class min + mean ----
    with (
        tc.tile_pool(name="seTp", bufs=1) as seT_pool,
        tc.tile_pool(name="mk", bufs=1) as mk_pool,
        tc.tile_pool(name="pd", bufs=7, space="PSUM") as pd_pool,
        tc.tile_pool(name="po", bufs=1, space="PSUM") as po_pool,
        tc.tile_pool(name="outs", bufs=1) as outs_pool,
    ):
        seT = []
        for j in range(8):
            t_ = seT_pool.tile([128, NSR], f16, tag=f"seT{j}", name=f"seT{j}")
            seT.append(t_)
            nc.sync.dma_start(
                out=t_[:].rearrange("p (c f) -> p c f", c=N_CORES),
                in_=ag_out[:, j * 128:(j + 1) * 128, 0:SPC]
                .rearrange("c p f -> p c f"))
        nc.sync.dma_start(
            out=ssq2[:].rearrange("p (c f) -> p c f", c=N_CORES),
            in_=ag_out[:, D_OUT:D_OUT + 2, 0:SPC].rearrange("c p f -> p c f"))

        for mt in range(MT):
            nc.vector.memset(mins[mt][:], 0.0)

        mkt = []
        for mt in range(MT):
            t_ = mk_pool.tile([128, QPC], f32, tag=f"mk{mt}", name=f"mk{mt}")
            mkt.append(t_)
            nc.gpsimd.dma_start(out=t_[:],
                                in_=mmask[mt * 128:(mt + 1) * 128, :])

        for mt in range(MT):
            mw = min(128, RPC - mt * 128)
            msl = slice(mt * 128, mt * 128 + mw)
            for ch in range(WAY):
                nsl = slice(ch * CLS, (ch + 1) * CLS)
                pd = pd_pool.tile([128, CLS], f32, tag="pd", name="pd")
                for j in range(8):
                    nc.tensor.matmul(
                        pd[:mw, :],
                        qacc16[j][:, msl],
                        seT[j][:, nsl],
                        start=(j == 0), stop=False,
                    )
                nc.tensor.matmul(pd[:mw, :], ones2[:, msl],
                                 ssq2[:, nsl], start=False, stop=True)
                nc.vector.tensor_reduce(
                    mins[mt][:mw, ch:ch + 1], pd[:mw, :],
                    axis=AX.X, op=ALU.min)
            # d2 = min(-2 q.se + s_sq) + q_sq, clamped at 0, then sqrt
            nc.vector.tensor_scalar(mins[mt][:mw, :], mins[mt][:mw, :],
                                    qsq_cols[mt][:mw, :], 0.0,
                                    ALU.add, ALU.max)
            nc.scalar.activation(mins[mt][:], mins[mt][:], AF.Sqrt)

        po = po_pool.tile([QPC, WAY], f32, tag="po", name="po")
        for mt in range(MT):
            nc.tensor.matmul(po[:], mkt[mt][:], mins[mt][:],
                             start=(mt == 0), stop=(mt == MT - 1))
        out_s = outs_pool.tile([QPC, WAY], f32, tag="out_s", name="out_s")
        nc.vector.tensor_copy(out_s[:], po[:])
        nc.sync.dma_start(out=out[:], in_=out_s[:])

    dram_ctx.__exit__(None, None, None)
    pre_ctx.__exit__(None, None, None)
    persist_ctx.__exit__(None, None, None)


_NC_CACHE = {}


def _get_nc():
    if "nc" not in _NC_CACHE:
        _NC_CACHE["nc"] = _build_nc()
    return _NC_CACHE["nc"]


def make_in_maps(support_set, support_labels, queries, clsW_w, clsW_b):
    support_set = np.asarray(support_set, dtype=np.float32)
    support_labels = np.asarray(support_labels)
    queries = np.asarray(queries, dtype=np.float32)
    clsW_w = np.asarray(clsW_w, dtype=np.float32)
    clsW_b = np.asarray(clsW_b, dtype=np.float32)

    # class-sort support rows so each class is a contiguous 280-column block
    perm = np.argsort(support_labels, kind="stable")
    S = support_set[perm].reshape(NSR, D_IN)

    STa = np.empty((D_IN + 1, NSR), np.float16)
    STa[:D_IN] = S.T.astype(np.float16)
    STa[D_IN] = 1.0

    Qp = np.zeros((NQR, D_IN), np.float32)
    Qp[:N_Q * T] = queries.reshape(N_Q * T, D_IN)
    QTa = np.empty((D_IN + 1, NQR), np.float16)
    QTa[:D_IN] = Qp.T.astype(np.float16)
    QTa[D_IN] = 1.0

    WTa = np.empty((D_IN + 1, D_OUT), np.float16)
    WTa[:D_IN] = clsW_w.T.astype(np.float16)
    WTa[D_IN] = clsW_b.astype(np.float16)

    mmask = np.zeros((MT * 128, QPC), np.float32)
    r = np.arange(RPC)
    mmask[r, r // T] = -1.0 / T

    ident = np.eye(128, dtype=np.float32)
    onesd = np.ones((128, NSR), np.float16)

    in_maps = []
    for c in range(N_CORES):
        in_maps.append({
            "qT": np.ascontiguousarray(QTa[:, c * RPC:(c + 1) * RPC]),
            "wT": WTa,
            "sT": np.ascontiguousarray(STa[:, c * SPC:(c + 1) * SPC]),
            "mmask": mmask,
            "ident": ident,
            "onesd": onesd,
        })
    return in_maps


def kernel(support_set, support_labels, queries, clsW_w, clsW_b):
    in_maps = make_in_maps(support_set, support_labels, queries, clsW_w,
                           clsW_b)
    nc = _get_nc()
    res = run_bass_kernel_spmd(nc, in_maps, list(range(N_CORES)))
    out = np.concatenate([res.results[c]["out"] for c in range(N_CORES)], 0)
    return np.ascontiguousarray(out[:N_Q]).astype(np.float32)



# revision 2
# speedup vs baseline: 1.3181x; 1.3181x over previous
"""Trainium2 Bass kernel for nn_DistanceLoss (retrieval_knn, 5-way 5-shot).

v2: full-fp8 (e4m3) DoubleRow rewrite.

Computation (per reference):
    q  = relu(queries.flat @ W.T + b)          [5600, 1024]
    se = relu(support.flat @ W.T + b)          [1400, 1024]
    d2 = q_sq + s_sq - 2 q @ se.T              [5600, 1400]
    out[q, c] = -mean_t min_{j in class c} sqrt(relu(d2))

Sharding (8 cores):
  - data-parallel over queries: 13 queries (728 rows) per core (padded 100->104)
  - support projection sharded by support cols (175/core), AllGathered (fp8)

Key points vs v1:
  - all big matmuls are fp8 e4m3 with MatmulPerfMode.DoubleRow (2 k-tiles of
    128 per instruction, 2x PE throughput). W is scaled by 64 on host so its
    (std 1/sqrt(6144)) entries land in fp8 normal range; the 1/64 is folded
    into the activation scale.
  - W, Q, S fully resident in SBUF (fp8 halves footprints); every projection
    accumulates its full K=6144 in PSUM in one sweep -> no fp32 SBUF
    accumulators, no vector adds.
  - support projection computed directly in transposed layout
    seT[dout, row] = W @ S (no PE transposes); bias folded via the ACT
    per-partition bias operand, relu+scale fused in the same ACT.
  - s_sq is shipped inside the fp8 AllGather payload as a 4-way fp8
    hi/mid/lo/lo2 split of -s_sq (the fold matmul multiplies by a -1 vector,
    K=4).
  - d2 sign game: matmul computes M = 2*qh.sh - s_sq, min_dist^2 =
    q_sq - max_c M, so the DVE reduce is a max and sqrt(relu(.)) becomes
    sqrt(-min(M - q_sq, 0)) via ACT Sqrt with scale=-1.
"""

import os
import sys

if "/opt/trn_rl_repo" not in sys.path:
    sys.path.insert(0, "/opt/trn_rl_repo")

import ml_dtypes
import numpy as np

import concourse.bacc as bacc
import concourse.mybir as mybir
import concourse.tile as tile
from concourse.bass_utils import run_bass_kernel_spmd

WAY, SHOT, T = 5, 5, 56
D_IN, D_OUT = 6144, 1024
N_Q, N_S = 100, 25
N_CORES = 8
QPC = 13                 # queries per core (104 padded)
RPC = QPC * T            # 728 query rows per core
NQR = N_CORES * RPC      # 5824 padded query rows
NSR = N_S * T            # 1400 support rows
SPC = NSR // N_CORES     # 175 support rows per core
SPCP = 176               # padded: total AG payload (1028*176 B) is 64B-mult
KP = D_IN // 256         # 24 k-pairs (DoubleRow: 2x128 contraction each)
NCH = RPC // 2           # 364 query-row matmul chunk
RPCP = 768               # q8 row stride (64B-aligned for dual-fp8 ldweights)
CLS = NSR // WAY         # 280 columns per class
MT = (RPC + 127) // 128  # 6 row tiles (5x128 + 88)
WSCALE = 64.0            # host multiplies W by this before fp8 cast

f32 = mybir.dt.float32
f16 = mybir.dt.float16
f8 = mybir.dt.float8e4
AF = mybir.ActivationFunctionType
ALU = mybir.AluOpType
AX = mybir.AxisListType
DR = mybir.MatmulPerfMode.DoubleRow

_MODE = os.environ.get("KERNEL_MODE", "full")


def _build_nc():
    nc = bacc.Bacc("TRN2", target_bir_lowering=False, debug=False,
                   num_devices=N_CORES)
    qT = nc.dram_tensor("qT", [D_IN, RPC], f8, kind="ExternalInput")
    wT = nc.dram_tensor("wT", [D_IN, D_OUT], f8, kind="ExternalInput")
    sT = nc.dram_tensor("sT", [D_IN, SPC], f8, kind="ExternalInput")
    bq = nc.dram_tensor("bq", [128, 8], f32, kind="ExternalInput")
    bs = nc.dram_tensor("bs", [128, 8], f32, kind="ExternalInput")
    mmask = nc.dram_tensor("mmask", [MT * 128, QPC], f32, kind="ExternalInput")
    ones16 = nc.dram_tensor("ones16", [128, 1], f16, kind="ExternalInput")
    negones = nc.dram_tensor("negones", [4, 128], f8, kind="ExternalInput")
    out = nc.dram_tensor("out", [QPC, WAY], f32, kind="ExternalOutput")

    with tile.TileContext(nc) as tc:
        _body(tc, nc, qT, wT, sT, bq, bs, mmask, ones16, negones, out)
    nc.finalize()
    return nc


def _body(tc, nc, qT, wT, sT, bq, bs, mmask, ones16, negones, out):
    persist_ctx = tc.tile_pool(name="persist", bufs=1)
    persist = persist_ctx.__enter__()

    def ptile(shape, name, dtype=f32):
        return persist.tile(shape, dtype, tag=name, name=name)

    # ---- persistent tiles ----
    wp = [ptile([128, 2, D_OUT], f"wp{g}", f8) for g in range(KP)]
    sp = [ptile([128, 2, SPC], f"sp{g}", f8) for g in range(KP)]
    qp = [ptile([128, 2, RPC], f"qp{g}", f8) for g in range(KP)]
    q8 = ptile([128, 8, RPCP], "q8", f8)        # relu'd fp8 query activations
    sqt = ptile([128, 8, RPC], "sqt", f16)     # q8^2 (exact in fp16)
    seP = ptile([128, 8, NSR], "seP", f8)      # gathered 2*se.T (fp8)
    nssq4 = ptile([4, NSR], "nssq4", f8)       # gathered s_sq 4-way fp8 split
    seL = ptile([128, 8, SPCP], "seL", f8)     # local 2*se.T, AG staging
    nst = [ptile([1, SPCP], f"nst{i}", f8) for i in range(4)]
    bqc = ptile([128, 8], "bqc")
    bsc = ptile([128, 8], "bsc")
    onec = ptile([128, 1], "onec", f16)
    nones = ptile([4, 128], "nones", f8)
    qsq_cols = [ptile([128, 1], f"qsqc{mt}") for mt in range(MT)]
    mins = [ptile([128, WAY], f"mins{mt}") for mt in range(MT)]
    mkt = [ptile([128, QPC], f"mk{mt}") for mt in range(MT)]
    sqs8 = ptile([128, 8, SPC], "sqs8", f16)   # (seL/2)^2 for s_sq
    # s_sq split scratch (fp32 rows); split stores s_sq/8 (fp8 max is ~240)
    s8row = ptile([1, SPC], "s8row")
    srow = [ptile([1, SPC], f"srow{i}") for i in range(3)]
    scast = [ptile([1, SPC], f"scast{i}") for i in range(3)]

    # ---- DMA: small constants on the gpsimd queue ----
    for mt in range(MT):
        nc.gpsimd.dma_start(out=mkt[mt][:],
                            in_=mmask[mt * 128:(mt + 1) * 128, :])
    nc.gpsimd.dma_start(out=bqc[:], in_=bq[:])
    nc.gpsimd.dma_start(out=bsc[:], in_=bs[:])
    nc.gpsimd.dma_start(out=onec[:], in_=ones16[:])
    nc.gpsimd.dma_start(out=nones[:], in_=negones[:])

    # ---- DMA: big streams on the sync queue (support-critical first) ----
    for g in range(KP):
        nc.sync.dma_start(
            out=wp[g][:],
            in_=wT[g * 256:(g + 1) * 256, :]
            .rearrange("(g p) d -> p g d", p=128))
        nc.sync.dma_start(
            out=sp[g][:],
            in_=sT[g * 256:(g + 1) * 256, :]
            .rearrange("(g p) d -> p g d", p=128))
    for g in range(KP):
        nc.sync.dma_start(
            out=qp[g][:],
            in_=qT[g * 256:(g + 1) * 256, :]
            .rearrange("(g p) d -> p g d", p=128))

    # ---- memsets (pad cols must be finite for the collective) ----
    nc.vector.memset(seL[:], 0.0)
    for i in range(4):
        nc.vector.memset(nst[i][:], 0.0)
    for mt in range(MT):
        nc.vector.memset(mins[mt][:], 0.0)

    # ---- allgather buffers ----
    dram_ctx = tc.tile_pool(name="dram", bufs=1, space="DRAM")
    dram = dram_ctx.__enter__()
    ag_in = dram.tile([D_OUT + 4, SPCP], f8, tag="ag_in", name="ag_in")
    ag_out = dram.tile([N_CORES, D_OUT + 4, SPCP], f8, tag="ag_out",
                       name="ag_out",
                       addr_space="Local" if _MODE == "nocc" else "Shared")

    # ---- phase A: support projection seT = W @ S, direct layout ----
    with (
        tc.tile_pool(name="psA", bufs=3, space="PSUM") as psA,
        tc.tile_pool(name="psS", bufs=1, space="PSUM") as psS,
    ):
        for j in range(8):
            jsl = slice(j * 128, (j + 1) * 128)
            ps = psA.tile([128, SPC], f32, tag="psA", name="psA")
            for g in range(KP):
                nc.tensor.matmul(
                    ps[:],
                    wp[g][:, :, jsl],
                    sp[g][:],
                    start=(g == 0), stop=(g == KP - 1),
                    perf_mode=DR,
                )
            # seL = relu(2*(z + b)) = 2*relu(z+b); psum holds 64*z
            nc.scalar.activation(seL[:, j, 0:SPC], ps[:], AF.Relu,
                                 bias=bsc[:, j:j + 1], scale=2.0 / WSCALE)
            # sh = seL/2 exactly; sq = sh^2 (exact in fp16)
            nc.scalar.activation(sqs8[:, j, :], seL[:, j, 0:SPC], AF.Square,
                                 scale=0.5)
            nc.gpsimd.dma_start(out=ag_in[j * 128:(j + 1) * 128, :],
                                in_=seL[:, j, :])

        # s_sq row via ones.T @ sq (sum over dout partitions)
        sps = psS.tile([1, SPC], f32, tag="ssq", name="ssq")
        for j in range(8):
            nc.tensor.matmul(sps[:], onec[:], sqs8[:, j, :],
                             start=(j == 0), stop=(j == 7))
        # 4-way fp8 split of s_sq/8 (fold matmul multiplies by -8)
        nc.vector.tensor_scalar_mul(s8row[:], sps[:], 0.125)
        prev = s8row
        for i in range(4):
            nc.vector.tensor_copy(nst[i][0:1, 0:SPC], prev[:])
            if i < 3:
                nc.vector.tensor_copy(scast[i][:], nst[i][0:1, 0:SPC])
                nc.vector.tensor_sub(srow[i][:], prev[:], scast[i][:])
                prev = srow[i]
        for i in range(4):
            nc.gpsimd.dma_start(out=ag_in[D_OUT + i:D_OUT + i + 1, :],
                                in_=nst[i][:])

        if _MODE == "nocc":
            for c in range(N_CORES):
                nc.gpsimd.dma_start(out=ag_out[c], in_=ag_in[:])
        else:
            nc.gpsimd.collective_compute(
                "AllGather",
                ALU.bypass,
                replica_groups=[list(range(N_CORES))],
                ins=[ag_in[:]],
                outs=[ag_out[:]],
            )

    # ---- merge DMAs (enqueue early on gpsimd queue; they wait on the AG) --
    for j in range(8):
        nc.gpsimd.dma_start(
            out=seP[:, j, :].rearrange("p (c f) -> p c f", c=N_CORES),
            in_=ag_out[:, j * 128:(j + 1) * 128, 0:SPC]
            .rearrange("c p f -> p c f"))
    nc.gpsimd.dma_start(
        out=nssq4[:].rearrange("p (c f) -> p c f", c=N_CORES),
        in_=ag_out[:, D_OUT:D_OUT + 4, 0:SPC].rearrange("c p f -> p c f"))

    # ---- phase B: query projection, transposed layout ----
    with tc.tile_pool(name="psB", bufs=4, space="PSUM") as psB:
        for m in range(8):
            msl = slice(m * 128, (m + 1) * 128)
            pstiles = [psB.tile([128, NCH], f32, tag="psB", name="psB")
                       for _ in range(2)]
            for g in range(KP):
                for n in range(2):
                    nsl = slice(n * NCH, (n + 1) * NCH)
                    nc.tensor.matmul(
                        pstiles[n][:],
                        wp[g][:, :, msl],
                        qp[g][:, :, nsl],
                        start=(g == 0), stop=(g == KP - 1),
                        perf_mode=DR,
                    )
            for n in range(2):
                nsl = slice(n * NCH, (n + 1) * NCH)
                nc.scalar.activation(q8[:, m, nsl], pstiles[n][:], AF.Relu,
                                     bias=bqc[:, m:m + 1], scale=1.0 / WSCALE)
                nc.scalar.activation(sqt[:, m, nsl], q8[:, m, nsl], AF.Square)

    # ---- q_sq columns: qsq[mt][r] = sum_dout q8^2 via sqt.T @ ones ----
    with tc.tile_pool(name="pqsqc", bufs=2, space="PSUM") as pqsqc:
        for mt in range(MT):
            mw = min(128, RPC - mt * 128)
            msl = slice(mt * 128, mt * 128 + mw)
            pq1 = pqsqc.tile([128, 1], f32, tag="pqsqc", name="pqsqc")
            for j in range(8):
                nc.tensor.matmul(pq1[:mw, :], sqt[:, j, msl], onec[:],
                                 start=(j == 0), stop=(j == 7))
            nc.vector.tensor_copy(qsq_cols[mt][:mw, :], pq1[:mw, :])

    # ---- phase D: distance + per-class max + mean ----
    with (
        tc.tile_pool(name="pd", bufs=6, space="PSUM") as pd_pool,
        tc.tile_pool(name="po", bufs=1, space="PSUM") as po_pool,
        tc.tile_pool(name="outs", bufs=1) as outs_pool,
    ):
        for mt in range(MT):
            mw = min(128, RPC - mt * 128)
            msl = slice(mt * 128, mt * 128 + mw)
            for ch in range(WAY):
                nsl = slice(ch * CLS, (ch + 1) * CLS)
                pd = pd_pool.tile([128, CLS], f32, tag="pd", name="pd")
                for jp in range(4):
                    nc.tensor.matmul(
                        pd[:mw, :],
                        q8[:, 2 * jp:2 * jp + 2, msl],
                        seP[:, 2 * jp:2 * jp + 2, nsl],
                        start=(jp == 0), stop=False,
                        perf_mode=DR,
                    )
                nc.tensor.matmul(pd[:mw, :], nones[:, :mw],
                                 nssq4[:, nsl], start=False, stop=True)
                nc.vector.tensor_reduce(
                    mins[mt][:mw, ch:ch + 1], pd[:mw, :],
                    axis=AX.X, op=ALU.max)
            # min d2 = q_sq - max M; d = sqrt(relu(.)) = sqrt(-min(M-q_sq,0))
            nc.vector.tensor_scalar(mins[mt][:mw, :], mins[mt][:mw, :],
                                    qsq_cols[mt][:mw, :], 0.0,
                                    ALU.subtract, ALU.min)
            nc.scalar.activation(mins[mt][:], mins[mt][:], AF.Sqrt,
                                 scale=-1.0)

        po = po_pool.tile([QPC, WAY], f32, tag="po", name="po")
        for mt in range(MT):
            nc.tensor.matmul(po[:], mkt[mt][:], mins[mt][:],
                             start=(mt == 0), stop=(mt == MT - 1))
        out_s = outs_pool.tile([QPC, WAY], f32, tag="out_s", name="out_s")
        nc.vector.tensor_copy(out_s[:], po[:])
        nc.gpsimd.dma_start(out=out[:], in_=out_s[:])

    dram_ctx.__exit__(None, None, None)
    persist_ctx.__exit__(None, None, None)


_NC_CACHE = {}


def _get_nc():
    if "nc" not in _NC_CACHE:
        _NC_CACHE["nc"] = _build_nc()
    return _NC_CACHE["nc"]


F8NP = ml_dtypes.float8_e4m3


def make_in_maps(support_set, support_labels, queries, clsW_w, clsW_b):
    support_set = np.asarray(support_set, dtype=np.float32)
    support_labels = np.asarray(support_labels)
    queries = np.asarray(queries, dtype=np.float32)
    clsW_w = np.asarray(clsW_w, dtype=np.float32)
    clsW_b = np.asarray(clsW_b, dtype=np.float32)

    # class-sort support rows so each class is a contiguous 280-column block
    perm = np.argsort(support_labels, kind="stable")
    S = support_set[perm].reshape(NSR, D_IN)
    STa = np.ascontiguousarray(S.T).astype(F8NP)      # [D_IN, NSR]

    Qp = np.zeros((NQR, D_IN), np.float32)
    Qp[:N_Q * T] = queries.reshape(N_Q * T, D_IN)
    QTa = np.ascontiguousarray(Qp.T).astype(F8NP)     # [D_IN, NQR]

    WTa = np.ascontiguousarray(clsW_w.T * WSCALE).astype(F8NP)  # [D_IN, D_OUT]

    bqa = np.ascontiguousarray(clsW_b.reshape(8, 128).T)        # [128, 8]
    bsa = np.ascontiguousarray(bqa * 2.0)

    mmask = np.zeros((MT * 128, QPC), np.float32)
    r = np.arange(RPC)
    mmask[r, r // T] = -1.0 / T

    ones16a = np.ones((128, 1), np.float16)
    negonesa = np.full((4, 128), -8.0, F8NP)

    in_maps = []
    for c in range(N_CORES):
        in_maps.append({
            "qT": np.ascontiguousarray(QTa[:, c * RPC:(c + 1) * RPC]),
            "wT": WTa,
            "sT": np.ascontiguousarray(STa[:, c * SPC:(c + 1) * SPC]),
            "bq": bqa,
            "bs": bsa,
            "mmask": mmask,
            "ones16": ones16a,
            "negones": negonesa,
        })
    return in_maps


def kernel(support_set, support_labels, queries, clsW_w, clsW_b):
    in_maps = make_in_maps(support_set, support_labels, queries, clsW_w,
                           clsW_b)
    nc = _get_nc()
    res = run_bass_kernel_spmd(nc, in_maps, list(range(N_CORES)))
    out = np.concatenate([res.results[c]["out"] for c in range(N_CORES)], 0)
    return np.ascontiguousarray(out[:N_Q]).astype(np.float32)


# revision 3
# speedup vs baseline: 1.3624x; 1.0336x over previous
"""Trainium2 Bass kernel for nn_DistanceLoss (retrieval_knn, 5-way 5-shot).

v2: full-fp8 (e4m3) DoubleRow rewrite.

Computation (per reference):
    q  = relu(queries.flat @ W.T + b)          [5600, 1024]
    se = relu(support.flat @ W.T + b)          [1400, 1024]
    d2 = q_sq + s_sq - 2 q @ se.T              [5600, 1400]
    out[q, c] = -mean_t min_{j in class c} sqrt(relu(d2))

Sharding (8 cores):
  - data-parallel over queries: 13 queries (728 rows) per core (padded 100->104)
  - support projection sharded by support cols (175/core), AllGathered (fp8)

Key points vs v1:
  - all big matmuls are fp8 e4m3 with MatmulPerfMode.DoubleRow (2 k-tiles of
    128 per instruction, 2x PE throughput). W is scaled by 64 on host so its
    (std 1/sqrt(6144)) entries land in fp8 normal range; the 1/64 is folded
    into the activation scale.
  - W, Q, S fully resident in SBUF (fp8 halves footprints); every projection
    accumulates its full K=6144 in PSUM in one sweep -> no fp32 SBUF
    accumulators, no vector adds.
  - support projection computed directly in transposed layout
    seT[dout, row] = W @ S (no PE transposes); bias folded via the ACT
    per-partition bias operand, relu+scale fused in the same ACT.
  - s_sq is shipped inside the fp8 AllGather payload as a 4-way fp8
    hi/mid/lo/lo2 split of -s_sq (the fold matmul multiplies by a -1 vector,
    K=4).
  - d2 sign game: matmul computes M = 2*qh.sh - s_sq, min_dist^2 =
    q_sq - max_c M, so the DVE reduce is a max and sqrt(relu(.)) becomes
    sqrt(-min(M - q_sq, 0)) via ACT Sqrt with scale=-1.
"""

import os
import sys

if "/opt/trn_rl_repo" not in sys.path:
    sys.path.insert(0, "/opt/trn_rl_repo")

import ml_dtypes
import numpy as np

import concourse.bacc as bacc
import concourse.mybir as mybir
import concourse.tile as tile
from concourse.bass_utils import run_bass_kernel_spmd

WAY, SHOT, T = 5, 5, 56
D_IN, D_OUT = 6144, 1024
N_Q, N_S = 100, 25
N_CORES = 8
QPC = 13                 # queries per core (104 padded)
RPC = QPC * T            # 728 query rows per core
NQR = N_CORES * RPC      # 5824 padded query rows
NSR = N_S * T            # 1400 support rows
SPC = NSR // N_CORES     # 175 support rows per core
SPCP = 176               # padded: total AG payload (1028*176 B) is 64B-mult
KP = D_IN // 256         # 24 k-pairs (DoubleRow: 2x128 contraction each)
NCH = RPC // 2           # 364 query-row matmul chunk
RPCP = 768               # q8 row stride (64B-aligned for dual-fp8 ldweights)
CLS = NSR // WAY         # 280 columns per class
MT = (RPC + 127) // 128  # 6 row tiles (5x128 + 88)
WSCALE = 64.0            # host multiplies W by this before fp8 cast

f32 = mybir.dt.float32
f16 = mybir.dt.float16
f8 = mybir.dt.float8e4
AF = mybir.ActivationFunctionType
ALU = mybir.AluOpType
AX = mybir.AxisListType
DR = mybir.MatmulPerfMode.DoubleRow

_MODE = os.environ.get("KERNEL_MODE", "full")


def _build_nc():
    nc = bacc.Bacc("TRN2", target_bir_lowering=False, debug=False,
                   num_devices=N_CORES)
    qT = nc.dram_tensor("qT", [6, 128, 4, RPC, 2], f8, kind="ExternalInput")
    wT = nc.dram_tensor("wT", [12, 128, 4, D_OUT], f8, kind="ExternalInput")
    sT = nc.dram_tensor("sT", [6, 128, 4, SPC, 2], f8, kind="ExternalInput")
    bq = nc.dram_tensor("bq", [128, 8], f32, kind="ExternalInput")
    bs = nc.dram_tensor("bs", [128, 8], f32, kind="ExternalInput")
    mmask = nc.dram_tensor("mmask", [MT * 128, QPC], f32, kind="ExternalInput")
    ones16 = nc.dram_tensor("ones16", [128, 1], f16, kind="ExternalInput")
    negones = nc.dram_tensor("negones", [4, 128], f8, kind="ExternalInput")
    out = nc.dram_tensor("out", [QPC, WAY], f32, kind="ExternalOutput")

    with tile.TileContext(nc) as tc:
        _body(tc, nc, qT, wT, sT, bq, bs, mmask, ones16, negones, out)
    nc.finalize()
    return nc


def _body(tc, nc, qT, wT, sT, bq, bs, mmask, ones16, negones, out):
    persist_ctx = tc.tile_pool(name="persist", bufs=1)
    persist = persist_ctx.__enter__()

    def ptile(shape, name, dtype=f32):
        return persist.tile(shape, dtype, tag=name, name=name)

    # ---- persistent tiles ----
    w4 = [ptile([128, 4, D_OUT], f"w4_{t}", f8) for t in range(12)]
    sI = [ptile([128, 4, SPC, 2], f"sI{t}", f8) for t in range(6)]
    qI = [ptile([128, 4, RPC, 2], f"qI{t}", f8) for t in range(6)]
    q8 = ptile([128, 8, RPCP], "q8", f8)        # relu'd fp8 query activations
    sqt = ptile([128, 8, RPC], "sqt", f16)     # q8^2 (exact in fp16)
    sePI = ptile([128, 4, NSR, 2], "sePI", f8)  # gathered 2*se.T, pair-ilv
    nssq4 = ptile([4, NSR], "nssq4", f8)       # gathered s_sq 4-way fp8 split
    seLI = ptile([128, 4, SPCP, 2], "seLI", f8)  # local 2*se.T, pair-ilv
    nst = [ptile([1, 2 * SPCP], f"nst{i}", f8) for i in range(4)]
    bqc = ptile([128, 8], "bqc")
    bsc = ptile([128, 8], "bsc")
    onec = ptile([128, 1], "onec", f16)
    nones = ptile([4, 128], "nones", f8)
    qsq_cols = [ptile([128, 1], f"qsqc{mt}") for mt in range(MT)]
    mins = [ptile([128, WAY], f"mins{mt}") for mt in range(MT)]
    mkt = [ptile([128, QPC], f"mk{mt}") for mt in range(MT)]
    sqs8 = ptile([128, 8, SPC], "sqs8", f16)   # (seL/2)^2 for s_sq
    # s_sq split scratch (fp32 rows); split stores s_sq/8 (fp8 max is ~240)
    s8row = ptile([1, SPC], "s8row")
    srow = [ptile([1, SPC], f"srow{i}") for i in range(3)]
    scast = [ptile([1, SPC], f"scast{i}") for i in range(3)]

    # ---- DMA: small constants on the gpsimd queue ----
    for mt in range(MT):
        nc.gpsimd.dma_start(out=mkt[mt][:],
                            in_=mmask[mt * 128:(mt + 1) * 128, :])
    nc.gpsimd.dma_start(out=bqc[:], in_=bq[:])
    nc.gpsimd.dma_start(out=bsc[:], in_=bs[:])
    nc.gpsimd.dma_start(out=onec[:], in_=ones16[:])
    nc.gpsimd.dma_start(out=nones[:], in_=negones[:])

    # ---- DMA: big streams on the sync queue (support-critical first) ----
    for t in range(6):
        nc.sync.dma_start(out=w4[2 * t][:], in_=wT[2 * t])
        nc.sync.dma_start(out=w4[2 * t + 1][:], in_=wT[2 * t + 1])
        nc.sync.dma_start(out=sI[t][:], in_=sT[t])
    for t in range(6):
        nc.sync.dma_start(out=qI[t][:], in_=qT[t])

    # ---- memsets (pad cols must be finite for the collective) ----
    nc.vector.memset(seLI[:], 0.0)
    for i in range(4):
        nc.vector.memset(nst[i][:], 0.0)
    for mt in range(MT):
        nc.vector.memset(mins[mt][:], 0.0)

    # ---- allgather buffers ----
    dram_ctx = tc.tile_pool(name="dram", bufs=1, space="DRAM")
    dram = dram_ctx.__enter__()
    ag_in = dram.tile([516, 2 * SPCP], f8, tag="ag_in", name="ag_in")
    ag_out = dram.tile([N_CORES, 516, 2 * SPCP], f8, tag="ag_out",
                       name="ag_out",
                       addr_space="Local" if _MODE == "nocc" else "Shared")

    # ---- phase A: support projection seT = W @ S, direct layout ----
    with (
        tc.tile_pool(name="psA", bufs=3, space="PSUM") as psA,
        tc.tile_pool(name="psS", bufs=1, space="PSUM") as psS,
    ):
        for j in range(8):
            jsl = slice(j * 128, (j + 1) * 128)
            ps = psA.tile([128, SPC], f32, tag="psA", name="psA")
            for g in range(KP):
                nc.tensor.matmul(
                    ps[:],
                    w4[g // 2][:, (g % 2) * 2:(g % 2) * 2 + 2, jsl],
                    sI[g // 4][:, g % 4, :, :].rearrange("p n t -> p t n"),
                    start=(g == 0), stop=(g == KP - 1),
                    perf_mode=DR,
                )
            # seL = relu(2*(z + b)) = 2*relu(z+b); psum holds 64*z
            nc.scalar.activation(seLI[:, j // 2, 0:SPC, j % 2], ps[:], AF.Relu,
                                 bias=bsc[:, j:j + 1], scale=2.0 / WSCALE)
            # sh = seL/2 exactly; sq = sh^2 (exact in fp16)
            nc.scalar.activation(sqs8[:, j, :], seLI[:, j // 2, 0:SPC, j % 2],
                                 AF.Square, scale=0.5)
            if j % 2 == 1:
                jp = j // 2
                nc.gpsimd.dma_start(
                    out=ag_in[jp * 128:(jp + 1) * 128, :],
                    in_=seLI[:, jp, :, :].rearrange("p n t -> p (n t)"))

        # s_sq row via ones.T @ sq (sum over dout partitions)
        sps = psS.tile([1, SPC], f32, tag="ssq", name="ssq")
        for j in range(8):
            nc.tensor.matmul(sps[:], onec[:], sqs8[:, j, :],
                             start=(j == 0), stop=(j == 7))
        # 4-way fp8 split of s_sq/8 (fold matmul multiplies by -8)
        nc.vector.tensor_scalar_mul(s8row[:], sps[:], 0.125)
        prev = s8row
        for i in range(4):
            nc.vector.tensor_copy(nst[i][0:1, 0:SPC], prev[:])
            if i < 3:
                nc.vector.tensor_copy(scast[i][:], nst[i][0:1, 0:SPC])
                nc.vector.tensor_sub(srow[i][:], prev[:], scast[i][:])
                prev = srow[i]
        for i in range(4):
            nc.gpsimd.dma_start(out=ag_in[512 + i:513 + i, :],
                                in_=nst[i][:])

        if _MODE == "nocc":
            for c in range(N_CORES):
                nc.gpsimd.dma_start(out=ag_out[c], in_=ag_in[:])
        else:
            nc.gpsimd.collective_compute(
                "AllGather",
                ALU.bypass,
                replica_groups=[list(range(N_CORES))],
                ins=[ag_in[:]],
                outs=[ag_out[:]],
            )

    # ---- merge DMAs (enqueue early on gpsimd queue; they wait on the AG) --
    for c in range(N_CORES):
        nc.gpsimd.dma_start(
            out=sePI[:, :, c * SPC:(c + 1) * SPC, :],
            in_=ag_out[c, 0:512, 0:2 * SPC]
            .rearrange("(jp p) b -> p jp b", p=128))
    nc.gpsimd.dma_start(
        out=nssq4[:].rearrange("p (c f) -> p c f", c=N_CORES),
        in_=ag_out[:, 512:516, 0:SPC].rearrange("c p f -> p c f"))

    # ---- phase B: query projection, transposed layout ----
    with tc.tile_pool(name="psB", bufs=4, space="PSUM") as psB:
        for m in range(8):
            msl = slice(m * 128, (m + 1) * 128)
            pstiles = [psB.tile([128, NCH], f32, tag="psB", name="psB")
                       for _ in range(2)]
            for g in range(KP):
                for n in range(2):
                    nsl = slice(n * NCH, (n + 1) * NCH)
                    nc.tensor.matmul(
                        pstiles[n][:],
                        w4[g // 2][:, (g % 2) * 2:(g % 2) * 2 + 2, msl],
                        qI[g // 4][:, g % 4, nsl, :]
                        .rearrange("p n t -> p t n"),
                        start=(g == 0), stop=(g == KP - 1),
                        perf_mode=DR,
                    )
            for n in range(2):
                nsl = slice(n * NCH, (n + 1) * NCH)
                nc.scalar.activation(q8[:, m, nsl], pstiles[n][:], AF.Relu,
                                     bias=bqc[:, m:m + 1], scale=1.0 / WSCALE)
                nc.scalar.activation(sqt[:, m, nsl], q8[:, m, nsl], AF.Square)

    # ---- q_sq columns: qsq[mt][r] = sum_dout q8^2 via sqt.T @ ones ----
    with tc.tile_pool(name="pqsqc", bufs=2, space="PSUM") as pqsqc:
        for mt in range(MT):
            mw = min(128, RPC - mt * 128)
            msl = slice(mt * 128, mt * 128 + mw)
            pq1 = pqsqc.tile([128, 1], f32, tag="pqsqc", name="pqsqc")
            for j in range(8):
                nc.tensor.matmul(pq1[:mw, :], sqt[:, j, msl], onec[:],
                                 start=(j == 0), stop=(j == 7))
            nc.vector.tensor_copy(qsq_cols[mt][:mw, :], pq1[:mw, :])

    # ---- phase D: distance + per-class max + mean ----
    with (
        tc.tile_pool(name="pd", bufs=6, space="PSUM") as pd_pool,
        tc.tile_pool(name="po", bufs=1, space="PSUM") as po_pool,
        tc.tile_pool(name="outs", bufs=1) as outs_pool,
    ):
        for mt in range(MT):
            mw = min(128, RPC - mt * 128)
            msl = slice(mt * 128, mt * 128 + mw)
            for ch in range(WAY):
                nsl = slice(ch * CLS, (ch + 1) * CLS)
                pd = pd_pool.tile([128, CLS], f32, tag="pd", name="pd")
                for jp in range(4):
                    nc.tensor.matmul(
                        pd[:mw, :],
                        q8[:, 2 * jp:2 * jp + 2, msl],
                        sePI[:, jp, nsl, :].rearrange("p n t -> p t n"),
                        start=(jp == 0), stop=False,
                        perf_mode=DR,
                    )
                nc.tensor.matmul(pd[:mw, :], nones[:, :mw],
                                 nssq4[:, nsl], start=False, stop=True)
                nc.vector.tensor_reduce(
                    mins[mt][:mw, ch:ch + 1], pd[:mw, :],
                    axis=AX.X, op=ALU.max)
            # min d2 = q_sq - max M; d = sqrt(relu(.)) = sqrt(-min(M-q_sq,0))
            nc.vector.tensor_scalar(mins[mt][:mw, :], mins[mt][:mw, :],
                                    qsq_cols[mt][:mw, :], 0.0,
                                    ALU.subtract, ALU.min)
            nc.scalar.activation(mins[mt][:], mins[mt][:], AF.Sqrt,
                                 scale=-1.0)

        po = po_pool.tile([QPC, WAY], f32, tag="po", name="po")
        for mt in range(MT):
            nc.tensor.matmul(po[:], mkt[mt][:], mins[mt][:],
                             start=(mt == 0), stop=(mt == MT - 1))
        out_s = outs_pool.tile([QPC, WAY], f32, tag="out_s", name="out_s")
        nc.vector.tensor_copy(out_s[:], po[:])
        nc.gpsimd.dma_start(out=out[:], in_=out_s[:])

    dram_ctx.__exit__(None, None, None)
    persist_ctx.__exit__(None, None, None)


_NC_CACHE = {}


def _get_nc():
    if "nc" not in _NC_CACHE:
        _NC_CACHE["nc"] = _build_nc()
    return _NC_CACHE["nc"]


F8NP = ml_dtypes.float8_e4m3


def make_in_maps(support_set, support_labels, queries, clsW_w, clsW_b):
    support_set = np.asarray(support_set, dtype=np.float32)
    support_labels = np.asarray(support_labels)
    queries = np.asarray(queries, dtype=np.float32)
    clsW_w = np.asarray(clsW_w, dtype=np.float32)
    clsW_b = np.asarray(clsW_b, dtype=np.float32)

    # class-sort support rows so each class is a contiguous 280-column block
    perm = np.argsort(support_labels, kind="stable")
    S = support_set[perm].reshape(NSR, D_IN)
    STa = S.T.astype(F8NP)                            # [D_IN, NSR]
    # pair-interleaved blocked layout [6, 128, 4, n, 2]
    STi = np.ascontiguousarray(
        STa.reshape(6, 4, 2, 128, NSR).transpose(0, 3, 1, 4, 2))

    Qp = np.zeros((NQR, D_IN), np.float32)
    Qp[:N_Q * T] = queries.reshape(N_Q * T, D_IN)
    QTa = Qp.T.astype(F8NP)                           # [D_IN, NQR]
    QTi = np.ascontiguousarray(
        QTa.reshape(6, 4, 2, 128, NQR).transpose(0, 3, 1, 4, 2))

    WTa = (clsW_w.T * WSCALE).astype(F8NP)            # [D_IN, D_OUT]
    WTb = np.ascontiguousarray(
        WTa.reshape(12, 4, 128, D_OUT).transpose(0, 2, 1, 3))

    bqa = np.ascontiguousarray(clsW_b.reshape(8, 128).T)        # [128, 8]
    bsa = np.ascontiguousarray(bqa * 2.0)

    mmask = np.zeros((MT * 128, QPC), np.float32)
    r = np.arange(RPC)
    mmask[r, r // T] = -1.0 / T

    ones16a = np.ones((128, 1), np.float16)
    negonesa = np.full((4, 128), -8.0, F8NP)

    in_maps = []
    for c in range(N_CORES):
        in_maps.append({
            "qT": np.ascontiguousarray(QTi[:, :, :, c * RPC:(c + 1) * RPC]),
            "wT": WTb,
            "sT": np.ascontiguousarray(STi[:, :, :, c * SPC:(c + 1) * SPC]),
            "bq": bqa,
            "bs": bsa,
            "mmask": mmask,
            "ones16": ones16a,
            "negones": negonesa,
        })
    return in_maps


def kernel(support_set, support_labels, queries, clsW_w, clsW_b):
    in_maps = make_in_maps(support_set, support_labels, queries, clsW_w,
                           clsW_b)
    nc = _get_nc()
    res = run_bass_kernel_spmd(nc, in_maps, list(range(N_CORES)))
    out = np.concatenate([res.results[c]["out"] for c in range(N_CORES)], 0)
    return np.ascontiguousarray(out[:N_Q]).astype(np.float32)


# revision 4
# speedup vs baseline: 1.4459x; 1.0613x over previous
"""Trainium2 Bass kernel for nn_DistanceLoss (retrieval_knn, 5-way 5-shot).

v2: full-fp8 (e4m3) DoubleRow rewrite.

Computation (per reference):
    q  = relu(queries.flat @ W.T + b)          [5600, 1024]
    se = relu(support.flat @ W.T + b)          [1400, 1024]
    d2 = q_sq + s_sq - 2 q @ se.T              [5600, 1400]
    out[q, c] = -mean_t min_{j in class c} sqrt(relu(d2))

Sharding (8 cores):
  - data-parallel over queries: 13 queries (728 rows) per core (padded 100->104)
  - support projection sharded by support cols (175/core), AllGathered (fp8)

Key points vs v1:
  - all big matmuls are fp8 e4m3 with MatmulPerfMode.DoubleRow (2 k-tiles of
    128 per instruction, 2x PE throughput). W is scaled by 64 on host so its
    (std 1/sqrt(6144)) entries land in fp8 normal range; the 1/64 is folded
    into the activation scale.
  - W, Q, S fully resident in SBUF (fp8 halves footprints); every projection
    accumulates its full K=6144 in PSUM in one sweep -> no fp32 SBUF
    accumulators, no vector adds.
  - support projection computed directly in transposed layout
    seT[dout, row] = W @ S (no PE transposes); bias folded via the ACT
    per-partition bias operand, relu+scale fused in the same ACT.
  - s_sq is shipped inside the fp8 AllGather payload as a 4-way fp8
    hi/mid/lo/lo2 split of -s_sq (the fold matmul multiplies by a -1 vector,
    K=4).
  - d2 sign game: matmul computes M = 2*qh.sh - s_sq, min_dist^2 =
    q_sq - max_c M, so the DVE reduce is a max and sqrt(relu(.)) becomes
    sqrt(-min(M - q_sq, 0)) via ACT Sqrt with scale=-1.
"""

import os
import sys

if "/opt/trn_rl_repo" not in sys.path:
    sys.path.insert(0, "/opt/trn_rl_repo")

import ml_dtypes
import numpy as np

import concourse.bacc as bacc
import concourse.mybir as mybir
import concourse.tile as tile
from concourse.bass_utils import run_bass_kernel_spmd

WAY, SHOT, T = 5, 5, 56
D_IN, D_OUT = 6144, 1024
N_Q, N_S = 100, 25
N_CORES = 8
QPC = 13                 # queries per core (104 padded)
RPC = QPC * T            # 728 query rows per core
NQR = N_CORES * RPC      # 5824 padded query rows
NSR = N_S * T            # 1400 support rows
SPC = NSR // N_CORES     # 175 support rows per core
SPCP = 176               # padded: total AG payload (1028*176 B) is 64B-mult
KP = D_IN // 256         # 24 k-pairs (DoubleRow: 2x128 contraction each)
NCH = RPC // 2           # 364 query-row matmul chunk
RPCP = 768               # q8 row stride (64B-aligned for dual-fp8 ldweights)
CLS = NSR // WAY         # 280 columns per class
MT = (RPC + 127) // 128  # 6 row tiles (5x128 + 88)
WSCALE = 64.0            # host multiplies W by this before fp8 cast

f32 = mybir.dt.float32
f16 = mybir.dt.float16
f8 = mybir.dt.float8e4
AF = mybir.ActivationFunctionType
ALU = mybir.AluOpType
AX = mybir.AxisListType
DR = mybir.MatmulPerfMode.DoubleRow

_MODE = os.environ.get("KERNEL_MODE", "full")


def _build_nc():
    nc = bacc.Bacc("TRN2", target_bir_lowering=False, debug=False,
                   num_devices=N_CORES)
    qT = nc.dram_tensor("qT", [6, 128, 4, RPC, 2], f8, kind="ExternalInput")
    wT = nc.dram_tensor("wT", [12, 128, 4, D_OUT], f8, kind="ExternalInput")
    sT = nc.dram_tensor("sT", [6, 128, 4, SPC, 2], f8, kind="ExternalInput")
    bq = nc.dram_tensor("bq", [128, 8], f32, kind="ExternalInput")
    bs = nc.dram_tensor("bs", [128, 8], f32, kind="ExternalInput")
    mmask = nc.dram_tensor("mmask", [MT * 128, QPC], f32, kind="ExternalInput")
    ones16 = nc.dram_tensor("ones16", [128, 1], f16, kind="ExternalInput")
    negones = nc.dram_tensor("negones", [4, 128], f8, kind="ExternalInput")
    out = nc.dram_tensor("out", [QPC, WAY], f32, kind="ExternalOutput")

    with tile.TileContext(nc) as tc:
        _body(tc, nc, qT, wT, sT, bq, bs, mmask, ones16, negones, out)
    nc.finalize()
    return nc


def _body(tc, nc, qT, wT, sT, bq, bs, mmask, ones16, negones, out):
    persist_ctx = tc.tile_pool(name="persist", bufs=1)
    persist = persist_ctx.__enter__()

    def ptile(shape, name, dtype=f32):
        return persist.tile(shape, dtype, tag=name, name=name)

    # ---- persistent tiles ----
    w4 = [ptile([128, 4, D_OUT], f"w4_{t}", f8) for t in range(12)]
    sI = [ptile([128, 4, SPC, 2], f"sI{t}", f8) for t in range(6)]
    qI = [ptile([128, 4, RPC, 2], f"qI{t}", f8) for t in range(6)]
    q8 = ptile([128, 8, RPCP], "q8", f8)        # relu'd fp8 query activations
    sqt = ptile([128, 8, RPC], "sqt", f16)     # q8^2 (exact in fp16)
    sePI = ptile([128, 4, NSR, 2], "sePI", f8)  # gathered 2*se.T, pair-ilv
    nssq4 = ptile([4, NSR], "nssq4", f8)       # gathered s_sq 4-way fp8 split
    nsr1 = ptile([1, NSR], "nsr1")             # -s_sq row (f32)
    nsrB = ptile([128, NSR], "nsrB")           # broadcast -s_sq
    seLI = ptile([128, 4, SPCP, 2], "seLI", f8)  # local 2*se.T, pair-ilv
    nst = [ptile([1, 2 * SPCP], f"nst{i}", f8) for i in range(4)]
    bqc = ptile([128, 8], "bqc")
    bsc = ptile([128, 8], "bsc")
    onec = ptile([128, 1], "onec", f16)
    nones = ptile([4, 128], "nones", f8)
    qsq_cols = [ptile([128, 1], f"qsqc{mt}") for mt in range(MT)]
    mins = [ptile([128, WAY], f"mins{mt}") for mt in range(MT)]
    mkt = [ptile([128, QPC], f"mk{mt}") for mt in range(MT)]
    sqs8 = ptile([128, 8, SPC], "sqs8", f16)   # (seL/2)^2 for s_sq
    # s_sq split scratch (fp32 rows); split stores s_sq/8 (fp8 max is ~240)
    s8row = ptile([1, SPC], "s8row")
    srow = [ptile([1, SPC], f"srow{i}") for i in range(3)]
    scast = [ptile([1, SPC], f"scast{i}") for i in range(3)]

    # ---- DMA: small constants on the gpsimd queue ----
    for mt in range(MT):
        nc.gpsimd.dma_start(out=mkt[mt][:],
                            in_=mmask[mt * 128:(mt + 1) * 128, :])
    nc.gpsimd.dma_start(out=bqc[:], in_=bq[:])
    nc.gpsimd.dma_start(out=bsc[:], in_=bs[:])
    nc.gpsimd.dma_start(out=onec[:], in_=ones16[:])
    nc.gpsimd.dma_start(out=nones[:], in_=negones[:])

    # ---- DMA: big streams on the sync queue (support-critical first) ----
    for t in range(6):
        nc.sync.dma_start(out=w4[2 * t][:], in_=wT[2 * t])
        nc.sync.dma_start(out=w4[2 * t + 1][:], in_=wT[2 * t + 1])
        nc.sync.dma_start(out=sI[t][:], in_=sT[t])
    for t in range(6):
        nc.sync.dma_start(out=qI[t][:], in_=qT[t])

    # ---- memsets (pad cols must be finite for the collective) ----
    nc.vector.memset(seLI[:], 0.0)
    for i in range(4):
        nc.vector.memset(nst[i][:], 0.0)
    for mt in range(MT):
        nc.vector.memset(mins[mt][:], 0.0)

    # ---- allgather buffers ----
    dram_ctx = tc.tile_pool(name="dram", bufs=1, space="DRAM")
    dram = dram_ctx.__enter__()
    ag_in = dram.tile([516, 2 * SPCP], f8, tag="ag_in", name="ag_in")
    ag_out = dram.tile([N_CORES, 516, 2 * SPCP], f8, tag="ag_out",
                       name="ag_out",
                       addr_space="Local" if _MODE == "nocc" else "Shared")

    # ---- phase A: support projection seT = W @ S, direct layout ----
    with tc.tile_pool(name="psA", bufs=1, space="PSUM") as psA:
        pstA = [psA.tile([128, SPC], f32, tag=f"psA{j}", name=f"psA{j}")
                for j in range(8)]
        for g in range(KP):
            smov = sI[g // 4][:, g % 4, :, :].rearrange("p n t -> p t n")
            for j in range(8):
                jsl = slice(j * 128, (j + 1) * 128)
                nc.tensor.matmul(
                    pstA[j][:],
                    w4[g // 2][:, (g % 2) * 2:(g % 2) * 2 + 2, jsl],
                    smov,
                    start=(g == 0), stop=(g == KP - 1),
                    perf_mode=DR,
                )
        for j in range(8):
            # seL = relu(2*(z + b)) = 2*relu(z+b); psum holds 64*z
            nc.scalar.activation(seLI[:, j // 2, 0:SPC, j % 2], pstA[j][:],
                                 AF.Relu, bias=bsc[:, j:j + 1],
                                 scale=2.0 / WSCALE)
            # sh = seL/2 exactly; sq = sh^2 (exact in fp16)
            nc.scalar.activation(sqs8[:, j, :], seLI[:, j // 2, 0:SPC, j % 2],
                                 AF.Square, scale=0.5)
            if j % 2 == 1:
                jp = j // 2
                nc.gpsimd.dma_start(
                    out=ag_in[jp * 128:(jp + 1) * 128, :],
                    in_=seLI[:, jp, :, :].rearrange("p n t -> p (n t)"))

    # s_sq row via ones.T @ sq (sum over dout partitions)
    with tc.tile_pool(name="psS", bufs=1, space="PSUM") as psS:
        sps = psS.tile([1, SPC], f32, tag="ssq", name="ssq")
        for j in range(8):
            nc.tensor.matmul(sps[:], onec[:], sqs8[:, j, :],
                             start=(j == 0), stop=(j == 7))
        # 4-way fp8 split of s_sq/8 (fold matmul multiplies by -8)
        nc.vector.tensor_scalar_mul(s8row[:], sps[:], 0.125)
        prev = s8row
        for i in range(4):
            nc.vector.tensor_copy(nst[i][0:1, 0:SPC], prev[:])
            if i < 3:
                nc.vector.tensor_copy(scast[i][:], nst[i][0:1, 0:SPC])
                nc.vector.tensor_sub(srow[i][:], prev[:], scast[i][:])
                prev = srow[i]
        for i in range(4):
            nc.gpsimd.dma_start(out=ag_in[512 + i:513 + i, :],
                                in_=nst[i][:])

        if _MODE == "nocc":
            for c in range(N_CORES):
                nc.gpsimd.dma_start(out=ag_out[c], in_=ag_in[:])
        else:
            nc.gpsimd.collective_compute(
                "AllGather",
                ALU.bypass,
                replica_groups=[list(range(N_CORES))],
                ins=[ag_in[:]],
                outs=[ag_out[:]],
            )

    # ---- merge DMAs (enqueue early on gpsimd queue; they wait on the AG) --
    for c in range(N_CORES):
        nc.gpsimd.dma_start(
            out=sePI[:, :, c * SPC:(c + 1) * SPC, :],
            in_=ag_out[c, 0:512, 0:2 * SPC]
            .rearrange("(jp p) b -> p jp b", p=128))
    nc.gpsimd.dma_start(
        out=nssq4[:].rearrange("p (c f) -> p c f", c=N_CORES),
        in_=ag_out[:, 512:516, 0:SPC].rearrange("c p f -> p c f"))

    # ---- phase B: query projection, transposed layout ----
    with tc.tile_pool(name="psB", bufs=4, space="PSUM") as psB:
        for m in range(8):
            msl = slice(m * 128, (m + 1) * 128)
            pstiles = [psB.tile([128, NCH], f32, tag="psB", name="psB")
                       for _ in range(2)]
            for g in range(KP):
                for n in range(2):
                    nsl = slice(n * NCH, (n + 1) * NCH)
                    nc.tensor.matmul(
                        pstiles[n][:],
                        w4[g // 2][:, (g % 2) * 2:(g % 2) * 2 + 2, msl],
                        qI[g // 4][:, g % 4, nsl, :]
                        .rearrange("p n t -> p t n"),
                        start=(g == 0), stop=(g == KP - 1),
                        perf_mode=DR,
                    )
            for n in range(2):
                nsl = slice(n * NCH, (n + 1) * NCH)
                nc.scalar.activation(q8[:, m, nsl], pstiles[n][:], AF.Relu,
                                     bias=bqc[:, m:m + 1], scale=1.0 / WSCALE)
                nc.scalar.activation(sqt[:, m, nsl], q8[:, m, nsl], AF.Square)

    # ---- q_sq columns: qsq[mt][r] = sum_dout q8^2 via sqt.T @ ones ----
    with tc.tile_pool(name="pqsqc", bufs=2, space="PSUM") as pqsqc:
        for mt in range(MT):
            mw = min(128, RPC - mt * 128)
            msl = slice(mt * 128, mt * 128 + mw)
            pq1 = pqsqc.tile([128, 1], f32, tag="pqsqc", name="pqsqc")
            for j in range(8):
                nc.tensor.matmul(pq1[:mw, :], sqt[:, j, msl], onec[:],
                                 start=(j == 0), stop=(j == 7))
            nc.vector.tensor_copy(qsq_cols[mt][:mw, :], pq1[:mw, :])

    # -s_sq row = (-8 ones).T @ splits, per class chunk (psum bank limit)
    with tc.tile_pool(name="psN", bufs=2, space="PSUM") as psN:
        for ch in range(WAY):
            nsl = slice(ch * CLS, (ch + 1) * CLS)
            psn = psN.tile([1, CLS], f32, tag="psn", name="psn")
            nc.tensor.matmul(psn[:], nones[:, 0:1], nssq4[:, nsl],
                             start=True, stop=True)
            nc.vector.tensor_copy(nsr1[0:1, nsl], psn[:])
    nc.gpsimd.partition_broadcast(nsrB[:], nsr1[:])

    # ---- phase D: distance + per-class max + mean ----
    with (
        tc.tile_pool(name="pd", bufs=6, space="PSUM") as pd_pool,
        tc.tile_pool(name="po", bufs=1, space="PSUM") as po_pool,
        tc.tile_pool(name="outs", bufs=1) as outs_pool,
    ):
        po = po_pool.tile([QPC, WAY], f32, tag="po", name="po")
        for mt in range(MT):
            mw = min(128, RPC - mt * 128)
            msl = slice(mt * 128, mt * 128 + mw)
            for ch in range(WAY):
                nsl = slice(ch * CLS, (ch + 1) * CLS)
                pd = pd_pool.tile([128, CLS], f32, tag="pd", name="pd")
                for jp in range(4):
                    nc.tensor.matmul(
                        pd[:mw, :],
                        q8[:, 2 * jp:2 * jp + 2, msl],
                        sePI[:, jp, nsl, :].rearrange("p n t -> p t n"),
                        start=(jp == 0), stop=(jp == 3),
                        perf_mode=DR,
                    )
                nc.vector.tensor_tensor(pd[:mw, :], pd[:mw, :],
                                        nsrB[:mw, nsl], op=ALU.add)
                nc.vector.tensor_reduce(
                    mins[mt][:mw, ch:ch + 1], pd[:mw, :],
                    axis=AX.X, op=ALU.max)
            # min d2 = q_sq - max M; d = sqrt(relu(.)) = sqrt(-min(M-q_sq,0))
            nc.vector.tensor_scalar(mins[mt][:mw, :], mins[mt][:mw, :],
                                    qsq_cols[mt][:mw, :], 0.0,
                                    ALU.subtract, ALU.min)
            nc.scalar.activation(mins[mt][:], mins[mt][:], AF.Sqrt,
                                 scale=-1.0)
            nc.tensor.matmul(po[:], mkt[mt][:], mins[mt][:],
                             start=(mt == 0), stop=(mt == MT - 1))

        out_s = outs_pool.tile([QPC, WAY], f32, tag="out_s", name="out_s")
        nc.vector.tensor_copy(out_s[:], po[:])
        nc.gpsimd.dma_start(out=out[:], in_=out_s[:])

    dram_ctx.__exit__(None, None, None)
    persist_ctx.__exit__(None, None, None)


_NC_CACHE = {}


def _get_nc():
    if "nc" not in _NC_CACHE:
        _NC_CACHE["nc"] = _build_nc()
    return _NC_CACHE["nc"]


F8NP = ml_dtypes.float8_e4m3


def make_in_maps(support_set, support_labels, queries, clsW_w, clsW_b):
    support_set = np.asarray(support_set, dtype=np.float32)
    support_labels = np.asarray(support_labels)
    queries = np.asarray(queries, dtype=np.float32)
    clsW_w = np.asarray(clsW_w, dtype=np.float32)
    clsW_b = np.asarray(clsW_b, dtype=np.float32)

    # class-sort support rows so each class is a contiguous 280-column block
    perm = np.argsort(support_labels, kind="stable")
    S = support_set[perm].reshape(NSR, D_IN)
    STa = S.T.astype(F8NP)                            # [D_IN, NSR]
    # pair-interleaved blocked layout [6, 128, 4, n, 2]
    STi = np.ascontiguousarray(
        STa.reshape(6, 4, 2, 128, NSR).transpose(0, 3, 1, 4, 2))

    Qp = np.zeros((NQR, D_IN), np.float32)
    Qp[:N_Q * T] = queries.reshape(N_Q * T, D_IN)
    QTa = Qp.T.astype(F8NP)                           # [D_IN, NQR]
    QTi = np.ascontiguousarray(
        QTa.reshape(6, 4, 2, 128, NQR).transpose(0, 3, 1, 4, 2))

    WTa = (clsW_w.T * WSCALE).astype(F8NP)            # [D_IN, D_OUT]
    WTb = np.ascontiguousarray(
        WTa.reshape(12, 4, 128, D_OUT).transpose(0, 2, 1, 3))

    bqa = np.ascontiguousarray(clsW_b.reshape(8, 128).T)        # [128, 8]
    bsa = np.ascontiguousarray(bqa * 2.0)

    mmask = np.zeros((MT * 128, QPC), np.float32)
    r = np.arange(RPC)
    mmask[r, r // T] = -1.0 / T

    ones16a = np.ones((128, 1), np.float16)
    negonesa = np.full((4, 128), -8.0, F8NP)

    in_maps = []
    for c in range(N_CORES):
        in_maps.append({
            "qT": np.ascontiguousarray(QTi[:, :, :, c * RPC:(c + 1) * RPC]),
            "wT": WTb,
            "sT": np.ascontiguousarray(STi[:, :, :, c * SPC:(c + 1) * SPC]),
            "bq": bqa,
            "bs": bsa,
            "mmask": mmask,
            "ones16": ones16a,
            "negones": negonesa,
        })
    return in_maps


def kernel(support_set, support_labels, queries, clsW_w, clsW_b):
    in_maps = make_in_maps(support_set, support_labels, queries, clsW_w,
                           clsW_b)
    nc = _get_nc()
    res = run_bass_kernel_spmd(nc, in_maps, list(range(N_CORES)))
    out = np.concatenate([res.results[c]["out"] for c in range(N_CORES)], 0)
    return np.ascontiguousarray(out[:N_Q]).astype(np.float32)
